# revision 26
# baseline (speedup 1.0000x reference)
"""TRN2 Bass kernel v3: masked MHA block (B=4, S=2048, C=768, H=12).

Sharding: 8 cores = 4 batches x 2 head-groups (6 heads each), collective-free;
host sums the two per-batch row-parallel partials and adds b_proj.

All matmuls bf16 (fp32 PSUM accum) -- fp8 softmax paths lose too much accuracy
because softmax-weight noise does NOT wash out relative to the attention
output.  Keys to speed (all measured on this part):
  - dense back-to-back PE streams run unthrottled at 2.4 GHz; sparse streams
    drop to the HAM cold state (half rate), so phase 2 is structured so the
    PE always has runnable matmuls (3 score-psum bufs, 2 av bufs, head-pair
    interleaving).
  - scores: two heads quadrant-packed via tile_position (109 ns/MM measured).
  - AV: 64-key subchunks packed the same way, ones column rides as softmax
    denominator row 64.
  - mask: additive fp8-DR identity matmul into the scores psum for MASK_PE_QBS
    q-blocks; multiplicative bf16 keep-mask on DVE (2x mode) for the rest.
  - exp: ACT activation(Exp)->bf16 for most (h, qb) units; DVE schraudolph
    tensor_scalar -> int16 bitcast bf16 for DVE_UNITS to balance engines.
"""

import math
from contextlib import ExitStack

import numpy as np

import concourse.tile as tile
from concourse import bacc, mybir
from concourse.bass_utils import run_bass_kernel_spmd

F32 = mybir.dt.float32
BF16 = mybir.dt.bfloat16
FP8E4 = mybir.dt.float8e4
FP8E5 = mybir.dt.float8e5
I16 = mybir.dt.int16
DR = mybir.MatmulPerfMode.DoubleRow

MASK_NEG = -57344.0
B, S, C, H = 4, 2048, 768, 12
HD = 64
H_PER_CORE = 6
D_CORE = H_PER_CORE * HD  # 384
QBLK = 512
KC = S // 128  # 16
QB = S // QBLK  # 4
ST = S // 128  # 16
N_CORES = 8
VSTRIDE = 80  # per-head vaug stride (65 used)

EXP_BIAS = -3.5  # only the schraudolph path bakes this in; cancels in softmax
A16 = 128.0 / math.log(2.0)
B16 = 16256.0 + A16 * EXP_BIAS - 8.1

import os

# (head, qb) units where exp runs on DVE (schraudolph) instead of ACT
if os.environ.get("V3_ACT_ONLY"):
    DVE_UNITS = set()
else:
    DVE_UNITS = {(h, qb) for h in (4, 5) for qb in range(QB)} - {
        (4, 0), (5, 0), (4, 1)}
# q-blocks whose mask is folded on the PE (fp8-DR identity); others use a
# multiplicative bf16 keep-mask on DVE after exp
if os.environ.get("V3_MASK_ALL_PE"):
    MASK_PE_QBS = set(range(QB))
elif os.environ.get("V3_MASK_PE_QB0"):
    MASK_PE_QBS = {0}
else:
    MASK_PE_QBS = set()


def _build_kernel():
    nc = bacc.Bacc(
        trn_type="TRN2", target_bir_lowering=False, debug=False, num_devices=N_CORES
    )

    xb_d = nc.dram_tensor("xb", [128, 6, S], BF16, kind="ExternalInput").ap()
    wq_d = nc.dram_tensor("wqb", [128, 6, D_CORE], BF16, kind="ExternalInput").ap()
    wk_d = nc.dram_tensor("wkb", [128, 6, D_CORE], BF16, kind="ExternalInput").ap()
    wv_d = nc.dram_tensor("wvb", [128, 6, D_CORE], BF16, kind="ExternalInput").ap()
    maske_d = nc.dram_tensor(
        "maske5", [128, KC, 2, S], FP8E5, kind="ExternalInput"
    ).ap()
    keep_d = nc.dram_tensor("keepb", [128, KC, S], BF16, kind="ExternalInput").ap()
    id8_d = nc.dram_tensor("ident8", [128, 2, 128], FP8E4, kind="ExternalInput").ap()
    wproj_d = nc.dram_tensor("wproj", [128, 3, C], BF16, kind="ExternalInput").ap()
    y_d = nc.dram_tensor("y", [S, C], BF16, kind="ExternalOutput").ap()

    with tile.TileContext(nc) as tc, ExitStack() as ctx:
        consts = ctx.enter_context(tc.tile_pool(name="consts", bufs=1))
        main = ctx.enter_context(tc.tile_pool(name="main", bufs=1))

        id8_sb = consts.tile([128, 2, 128], FP8E4, tag="id8", name="id8")
        nc.sync.dma_start(id8_sb[:], id8_d[:])
        wproj_sb = consts.tile([128, 3, C], BF16, tag="wproj", name="wproj")
        nc.sync.dma_start(wproj_sb[:], wproj_d[:])

        qT_sb = main.tile([128, 3, S], BF16, tag="qT", name="qT")
        kT_sb = main.tile([128, 3, S], BF16, tag="kT", name="kT")
        vaug = main.tile([128, KC, H_PER_CORE * VSTRIDE], BF16, tag="vaug", name="vaug")
        attn_sb = main.tile([128, 3, S], BF16, tag="attn", name="attn")

        vaug_h = vaug.rearrange("p kc (h u) -> p kc h u", u=VSTRIDE)
        nc.gpsimd.memset(vaug_h[:, :, :, HD : HD + 1], 1.0)

        mpool = ctx.enter_context(tc.tile_pool(name="mask", bufs=2))
        mask_cache = {}

        def load_mask(qb_i):
            if qb_i in MASK_PE_QBS:
                mh = mpool.tile([128, KC, 2, QBLK], FP8E5, tag="mask", name="maske")
                nc.sync.dma_start(
                    mh[:], maske_d[:, :, :, qb_i * QBLK : (qb_i + 1) * QBLK]
                )
            else:
                mh = mpool.tile([128, KC, QBLK], BF16, tag="mask", name="maskk")
                nc.sync.dma_start(
                    mh[:], keep_d[:, :, qb_i * QBLK : (qb_i + 1) * QBLK]
                )
            return mh

        mask_cache[0] = load_mask(0)
        if os.environ.get("V3_NO_AV") or os.environ.get("V3_NO_NORM"):
            nc.gpsimd.memset(attn_sb[:], 0.0)

        # ---------------- phase 1: QKV projections (bf16) ----------
        with ExitStack() as p1:
            xpool = p1.enter_context(tc.tile_pool(name="x1", bufs=1))
            wpool = p1.enter_context(tc.tile_pool(name="w1", bufs=1))
            ps1 = p1.enter_context(tc.tile_pool(name="ps1", bufs=5, space="PSUM"))
            psv1 = p1.enter_context(tc.tile_pool(name="psv1", bufs=2, space="PSUM"))

            xb_sb = xpool.tile([128, 6, S], BF16, tag="xb", name="xb")
            nc.sync.dma_start(xb_sb[:], xb_d[:])
            w_sbs = []
            for nm, w_ap in (("wq", wq_d), ("wk", wk_d), ("wv", wv_d)):
                w_sb = wpool.tile([128, 6, D_CORE], BF16, tag=nm, name=nm)
                nc.sync.dma_start(w_sb[:], w_ap[:])
                w_sbs.append(w_sb)
            wq_sb, wk_sb, wv_sb = w_sbs

            cp_i = 0
            for w_sb, dst in ((wq_sb, qT_sb), (wk_sb, kT_sb)):
                for m in range(3):
                    # k-outer: each weight chunk loads once and serves all
                    # four n-blocks back-to-back (cached-weight MM rate)
                    pss = [
                        ps1.tile([128, QBLK], F32, tag="psqk", name=f"psqk{nb}")
                        for nb in range(QB)
                    ]
                    for k in range(6):
                        for nb in range(QB):
                            nc.tensor.matmul(
                                pss[nb][:],
                                w_sb[:, k, m * 128 : (m + 1) * 128],
                                xb_sb[:, k, nb * QBLK : (nb + 1) * QBLK],
                                start=(k == 0),
                                stop=(k == 5),
                            )
                    for nb in range(QB):
                        dst_ap = dst[:, m, nb * QBLK : (nb + 1) * QBLK]
                        if cp_i % 2 == 0:
                            nc.vector.tensor_copy(dst_ap, pss[nb][:])
                        else:
                            nc.scalar.copy(dst_ap, pss[nb][:])
                        cp_i += 1

            for st in range(ST):
                psv = psv1.tile([128, D_CORE], F32, tag="psv", name="psv")
                for k in range(6):
                    nc.tensor.matmul(
                        psv[:],
                        xb_sb[:, k, st * 128 : (st + 1) * 128],
                        wv_sb[:, k, :],
                        start=(k == 0),
                        stop=(k == 5),
                    )
                dst = vaug_h[:, st, :, 0:HD]
                src = psv.rearrange("p (h d) -> p h d", d=HD)
                if st % 2 == 0:
                    nc.vector.tensor_copy(dst, src)
                else:
                    nc.scalar.copy(dst, src)

        # ---------------- phase 2: attention ----------------
        with ExitStack() as p2:
            ppool = p2.enter_context(tc.tile_pool(name="pT", bufs=3))
            dpool = p2.enter_context(tc.tile_pool(name="div", bufs=2))
            bpool = p2.enter_context(tc.tile_pool(name="bcast", bufs=2))
            ps_s = p2.enter_context(tc.tile_pool(name="ps_s", bufs=2, space="PSUM"))
            ps_av = p2.enter_context(tc.tile_pool(name="ps_av", bufs=4, space="PSUM"))

            for qb in range(QB):
                if qb + 1 < QB:
                    mask_cache[qb + 1] = load_mask(qb + 1)
                mask_sb = mask_cache.pop(qb)
                mask_on_pe = qb in MASK_PE_QBS
                tmp_all = bpool.tile(
                    [HD, 3, QBLK], F32, tag="tmp_all", name="tmp_all", bufs=2
                )

                for hp in range(3):
                    hA, hB = 2 * hp, 2 * hp + 1
                    pTs = {}
                    for h in (hA, hB):
                        pTs[h] = ppool.tile([128, KC, QBLK], BF16, tag="pT", name="pT")
                    # scores + mask + exp, kc-pair at a time, heads interleaved
                    for kcp in range(KC // 2):
                        scs = {}
                        for h in (hA, hB):
                            scs[h] = ps_s.tile(
                                [128, 2, QBLK], F32, tag="sc", name="sc"
                            )
                        for c in range(2):
                            kc = 2 * kcp + c
                            for h in (hA, hB):
                                row0 = (h % 2) * HD
                                nc.tensor.matmul(
                                    scs[h][:, c, :],
                                    kT_sb[
                                        row0 : row0 + HD, hp, kc * 128 : (kc + 1) * 128
                                    ],
                                    qT_sb[
                                        row0 : row0 + HD,
                                        hp,
                                        qb * QBLK : (qb + 1) * QBLK,
                                    ],
                                    start=True,
                                    stop=not mask_on_pe,
                                    tile_position=(row0, 0),
                                )
                            if mask_on_pe:
                                for h in (hA, hB):
                                    nc.tensor.matmul(
                                        scs[h][:, c, :],
                                        id8_sb[:],
                                        mask_sb[:, kc, :, :],
                                        start=False,
                                        stop=True,
                                        perf_mode=DR,
                                    )
                        for h in (hA, hB):
                            pslab = pTs[h][:, 2 * kcp : 2 * kcp + 2, :]
                            if (h, qb) in DVE_UNITS:
                                nc.vector.tensor_scalar(
                                    pslab.bitcast(I16),
                                    scs[h][:],
                                    A16,
                                    B16,
                                    mybir.AluOpType.mult,
                                    mybir.AluOpType.add,
                                )
                            else:
                                nc.scalar.activation(
                                    pslab,
                                    scs[h][:],
                                    mybir.ActivationFunctionType.Exp,
                                )
                            if not mask_on_pe and kcp % 2 == 1:
                                wide = pTs[h][:, 2 * kcp - 2 : 2 * kcp + 2, :]
                                nc.vector.tensor_mul(
                                    wide,
                                    wide,
                                    mask_sb[:, 2 * kcp - 2 : 2 * kcp + 2, :],
                                )
                    if os.environ.get("V3_NO_AV"):
                        continue
                    # AV: 64-key subchunks quadrant-packed across the pair;
                    # each psum tile is written from exactly one quadrant
                    # (low/high halves merged during normalization)
                    if not os.environ.get("V3_AV_PACKED"):
                        avs = {
                            hA: ps_av.tile([HD + 1, QBLK], F32, tag="av", name="avA"),
                            hB: ps_av.tile([HD + 1, QBLK], F32, tag="av", name="avB"),
                        }
                        for h in (hA, hB):
                            for kc in range(KC):
                                nc.tensor.matmul(
                                    avs[h][:],
                                    vaug_h[:, kc, h, 0 : HD + 1],
                                    pTs[h][:, kc, :],
                                    start=(kc == 0),
                                    stop=(kc == KC - 1),
                                )
                        avs2 = None
                    else:
                        avs = {
                            hA: ps_av.tile([HD + 1, QBLK], F32, tag="av", name="avAL"),
                            hB: ps_av.tile([HD + 1, QBLK], F32, tag="av", name="avBL"),
                        }
                        avs2 = {
                            hA: ps_av.tile([HD + 1, QBLK], F32, tag="av", name="avAH"),
                            hB: ps_av.tile([HD + 1, QBLK], F32, tag="av", name="avBH"),
                        }
                        for kc in range(KC):
                            for h, hofs in ((hA, 0), (hB, HD)):
                                rr = hofs  # low half for A first, high for B
                                tgt = avs if rr == 0 else avs2
                                nc.tensor.matmul(
                                    tgt[h][:],
                                    vaug_h[rr : rr + HD, kc, h, 0 : HD + 1],
                                    pTs[h][rr : rr + HD, kc, :],
                                    start=(kc == 0),
                                    stop=(kc == KC - 1),
                                    tile_position=(rr, 0),
                                )
                            for h, hofs in ((hA, HD), (hB, 0)):
                                rr = hofs
                                tgt = avs if rr == 0 else avs2
                                nc.tensor.matmul(
                                    tgt[h][:],
                                    vaug_h[rr : rr + HD, kc, h, 0 : HD + 1],
                                    pTs[h][rr : rr + HD, kc, :],
                                    start=(kc == 0),
                                    stop=(kc == KC - 1),
                                    tile_position=(rr, 0),
                                )
                    if os.environ.get("V3_NO_NORM"):
                        continue
                    # normalize: denominator rows -> partition 0 via DMA,
                    # reciprocal, partition-broadcast, scale
                    den = dpool.tile([65, 2, QBLK], F32, tag="den", name="den")
                    for i, h in ((0, hA), (1, hB)):
                        if avs2 is None:
                            nc.scalar.copy(den[64:65, i, :], avs[h][HD : HD + 1, :])
                        else:
                            nc.scalar.copy(
                                den[64:65, i, :], avs2[h][HD : HD + 1, :]
                            )
                            nc.vector.tensor_add(
                                den[64:65, i, :],
                                den[64:65, i, :],
                                avs[h][HD : HD + 1, :],
                            )
                    r2 = dpool.tile([1, 2, QBLK], F32, tag="r2", name="r2")
                    nc.gpsimd.dma_start(r2[:], den[64:65, :, :])
                    rec2 = dpool.tile([1, 2, QBLK], F32, tag="rec2", name="rec2")
                    nc.vector.reciprocal_approx_fast(rec2[:], r2[:])
                    for i, h in ((0, hA), (1, hB)):
                        bc = bpool.tile([HD, QBLK], F32, tag="bc", name="bc")
                        nc.gpsimd.partition_broadcast(bc[:], rec2[:, i, :])
                        if h % 2 == 0:
                            dst = attn_sb[:HD, hp, qb * QBLK : (qb + 1) * QBLK]
                        else:
                            dst = tmp_all[:, hp, :]
                        if avs2 is None:
                            nc.vector.tensor_mul(dst, avs[h][:HD, :], bc[:])
                        else:
                            avsum = bpool.tile(
                                [HD, QBLK], F32, tag="avsum", name="avsum"
                            )
                            nc.scalar.copy(avsum[:], avs2[h][:HD, :])
                            nc.vector.tensor_add(
                                avsum[:], avsum[:], avs[h][:HD, :]
                            )
                            nc.vector.tensor_mul(dst, avsum[:], bc[:])
                if not (os.environ.get("V3_NO_AV") or os.environ.get("V3_NO_NORM")):
                    nc.gpsimd.dma_start(
                        attn_sb[HD:128, :, qb * QBLK : (qb + 1) * QBLK], tmp_all[:]
                    )


        # ---------------- phase 3: output projection (bf16) ----------------
        with ExitStack() as p3:
            ypool = p3.enter_context(tc.tile_pool(name="y", bufs=3))
            ps_y = p3.enter_context(tc.tile_pool(name="ps_y", bufs=4, space="PSUM"))
            y_r = y_d.rearrange("(st p) o -> st p o", p=128)
            for st in range(ST):
                y_sb = ypool.tile([128, C], BF16, tag="ysb", name="y_sb")
                for nb2 in range(2):
                    ps = ps_y.tile([128, 384], F32, tag="psy", name="psy")
                    for k3 in range(3):
                        nc.tensor.matmul(
                            ps[:],
                            attn_sb[:, k3, st * 128 : (st + 1) * 128],
                            wproj_sb[:, k3, nb2 * 384 : (nb2 + 1) * 384],
                            start=(k3 == 0),
                            stop=(k3 == 2),
                        )
                    if nb2 == 0:
                        nc.vector.tensor_copy(y_sb[:, :384], ps[:])
                    else:
                        nc.scalar.copy(y_sb[:, 384:], ps[:])
                nc.sync.dma_start(y_r[st], y_sb[:])

    nc.compile()
    return nc


def _prep_core_inputs(x, mask, w_qkv, w_proj, core):
    import ml_dtypes

    f8 = ml_dtypes.float8_e4m3
    f8e5 = ml_dtypes.float8_e5m2
    bf16 = ml_dtypes.bfloat16

    b, g = core // 2, core % 2
    s0, s1 = D_CORE * g, D_CORE * (g + 1)

    def cmajor(a):  # [C, n] -> [128, 6, n]
        return np.ascontiguousarray(a.reshape(6, 128, a.shape[1]).transpose(1, 0, 2))

    xT = np.ascontiguousarray(x[b].T)  # [C, S]
    wq = w_qkv[s0:s1, :].T * (HD ** -0.5)
    wk = w_qkv[C + s0 : C + s1, :].T
    wv = w_qkv[2 * C + s0 : 2 * C + s1, :].T

    maskT = mask[b].T  # [S(key), S(q)]
    m2 = np.zeros((128, KC, 2, S), dtype=f8e5)
    m2[:, :, 0, :] = (
        np.array([0.0, MASK_NEG], dtype=np.float32)[maskT]
        .reshape(KC, 128, S)
        .transpose(1, 0, 2)
        .astype(f8e5)
    )
    keep = (
        np.array([1.0, 0.0], dtype=np.float32)[maskT]
        .reshape(KC, 128, S)
        .transpose(1, 0, 2)
        .astype(bf16)
    )

    id8 = np.zeros((128, 2, 128), dtype=f8)
    id8[:, 0, :] = np.eye(128, dtype=np.float32).astype(f8)

    wproj = np.ascontiguousarray(w_proj[:, s0:s1].T)  # [384, C]
    wproj2 = wproj.reshape(3, 128, C).transpose(1, 0, 2)

    return {
        "xb": cmajor(xT).astype(bf16),
        "wqb": cmajor(np.ascontiguousarray(wq)).astype(bf16),
        "wkb": cmajor(np.ascontiguousarray(wk)).astype(bf16),
        "wvb": cmajor(np.ascontiguousarray(wv)).astype(bf16),
        "maske5": m2,
        "keepb": np.ascontiguousarray(keep),
        "ident8": id8,
        "wproj": np.ascontiguousarray(wproj2).astype(bf16),
    }


_NC_CACHE = {}


def get_nc():
    if "nc" not in _NC_CACHE:
        _NC_CACHE["nc"] = _build_kernel()
    return _NC_CACHE["nc"]


def _build_runner(nc):
    """Reusable jitted shard_map callable over the 8 cores."""
    import jax
    from jax.experimental.shard_map import shard_map
    from jax.sharding import Mesh, PartitionSpec

    from concourse.bass2jax import (
        _bass_exec_p,
        install_neuronx_cc_hook,
        partition_id_tensor,
    )

    install_neuronx_cc_hook()
    partition_name = nc.partition_id_tensor.name if nc.partition_id_tensor else None
    in_names, out_names, out_avals, zero_outs = [], [], [], []
    for alloc in nc.m.functions[0].allocations:
        if not isinstance(alloc, mybir.MemoryLocationSet):
            continue
        name = alloc.memorylocations[0].name
        if alloc.kind == "ExternalInput":
            if name != partition_name:
                in_names.append(name)
        elif alloc.kind == "ExternalOutput":
            out_names.append(name)
            shape = tuple(alloc.tensor_shape)
            dtype = mybir.dt.np(alloc.dtype)
            out_avals.append(jax.core.ShapedArray(shape, dtype))
            zero_outs.append(np.zeros(shape, dtype))
    n_params = len(in_names)
    all_in_names = list(in_names) + list(out_names)
    if partition_name is not None:
        all_in_names.append(partition_name)

    def _body(*args):
        operands = list(args)
        if partition_name is not None:
            operands.append(partition_id_tensor())
        outs = _bass_exec_p.bind(
            *operands,
            out_avals=tuple(out_avals),
            in_names=tuple(all_in_names),
            out_names=tuple(out_names),
            lowering_input_output_aliases=(),
            sim_require_finite=True,
            sim_require_nnan=True,
            nc=nc,
        )
        return tuple(outs)

    n_cores = nc.num_devices
    devices = jax.devices()[:n_cores]
    mesh = Mesh(np.asarray(devices), ("core",))
    in_specs = (PartitionSpec("core"),) * (n_params + len(out_names))
    out_specs = (PartitionSpec("core"),) * len(out_names)
    fn = jax.jit(
        shard_map(
            _body, mesh=mesh, in_specs=in_specs, out_specs=out_specs, check_rep=False
        ),
        keep_unused=True,
    )
    return fn, in_names, out_names, zero_outs


_RUNNER_CACHE = {}


def get_runner(nc, in_maps):
    """Return (fn, dev_args) for repeated dispatch of `nc` with `in_maps`."""
    import jax
    from jax.sharding import Mesh, NamedSharding, PartitionSpec

    key = id(nc)
    if key not in _RUNNER_CACHE:
        _RUNNER_CACHE[key] = _build_runner(nc)
    fn, in_names, out_names, zero_outs = _RUNNER_CACHE[key]
    n_cores = nc.num_devices
    mesh = Mesh(np.asarray(jax.devices()[:n_cores]), ("core",))
    shard = NamedSharding(mesh, PartitionSpec("core"))
    concat_in = [
        np.concatenate([np.asarray(in_maps[c][n]) for c in range(n_cores)], axis=0)
        for n in in_names
    ]
    dev_in = [jax.device_put(a, shard) for a in concat_in]
    zkey = ("zeros", key)
    if zkey not in _RUNNER_CACHE:
        concat_zeros = [
            np.zeros((n_cores * z.shape[0], *z.shape[1:]), z.dtype) for z in zero_outs
        ]
        _RUNNER_CACHE[zkey] = [jax.device_put(a, shard) for a in concat_zeros]
    return fn, dev_in + _RUNNER_CACHE[zkey]


def run_cached(nc, in_maps):
    """Execute via the cached runner; returns per-core result dicts."""
    fn, dev_args = get_runner(nc, in_maps)
    out_arrs = fn(*dev_args)
    _, _, out_names, zero_outs = _RUNNER_CACHE[id(nc)]
    n_cores = nc.num_devices
    fetched = [
        np.asarray(a).reshape(n_cores, *zero_outs[i].shape)
        for i, a in enumerate(out_arrs)
    ]
    return [
        {name: fetched[i][c] for i, name in enumerate(out_names)}
        for c in range(n_cores)
    ]


def make_in_maps(x, mask, w_qkv, w_proj):
    return [_prep_core_inputs(x, mask, w_qkv, w_proj, c) for c in range(N_CORES)]


def combine(results, b_proj):
    outs = []
    for b in range(B):
        outs.append(results[2 * b]["y"] + results[2 * b + 1]["y"] + b_proj[None, :])
    return np.stack(outs).astype(np.float32)


def kernel(x, mask, w_qkv, w_proj, b_proj):
    x = np.asarray(x, dtype=np.float32)
    mask = np.asarray(mask)
    w_qkv = np.asarray(w_qkv, dtype=np.float32)
    w_proj = np.asarray(w_proj, dtype=np.float32)
    b_proj = np.asarray(b_proj, dtype=np.float32)

    nc = get_nc()
    in_maps = make_in_maps(x, mask, w_qkv, w_proj)
    try:
        results = run_cached(nc, in_maps)
    except Exception:
        results = run_bass_kernel_spmd(nc, in_maps, list(range(N_CORES))).results
    return combine(results, b_proj)
